# revision 13
# baseline (speedup 1.0000x reference)
"""Cross-attention (txt queries -> image kv) Trainium2 Bass kernel.

Sharding: data-parallel over batch — B=8 batches, one NeuronCore each.
Host-side prep: image columns are COMPACTED to valid kv positions (padded to
jp, a multiple of 256) and txt rows PERMUTED valid-first (attention runs on
the first ip columns only; outputs un-permuted on host). Invalid-q rows are
reconstructed exactly via the ymeanb blend (uniform attention over all kv).
Per core (batch b):
  Q^T = Wq^T T^T / 8          [e, i]   bf16 SBUF-resident
  K^T = Wk^T X^T              [e, j]   bf16 SBUF-resident
  V'  = (X Wv) * kvm_j        [j, e]   bf16 SBUF-resident; per head tiles
                                       [j, h, 65] whose col 64 holds kvm_j
                                       (so PV accumulates both numerator and
                                       softmax denominator with the kv mask
                                       applied exactly)
  S^T_h = K_h Q_h^T           [j, i]   psum, two heads per [128,2,512] tile
  P^T = exp(S^T * kvm_j)      (ACT scale=kvm; masked/padded rows give exp(0)=1
                               but are zeroed by V' — no separate mask op)
  O^T_aug_h = [V'_h | kvm] ^T P^T -> [65, i] psum accum over jchunks
  O^T = O^T_aug[0:64] * recip(denom) broadcast via PE ones-outer-product
  Y = O^T.T Wout; blend: y = qm_i*Y + (1-qm_i)*ymeanb + qm_i*bout
    (q_mask=False rows = uniform attention over all kv -> host-computed
     ymeanb = (mean_j X) @ Wv @ Wout + bout)

Fast path (jp <= 2816): X^T kept fully SBUF-resident — read from HBM exactly
once (the old streaming path re-read it 5x).  K^T chunk computation (PE) is
software-pipelined against the previous chunk's attention (QK^T/exp/PV), so
the Activation engine's exp work hides behind PE matmuls, and within the
attention inner loop QK^T(jc+1) is issued ahead of PV(jc) so PE never waits
on the exp of the current chunk.

All PE matmuls in bf16 (inputs pre-cast to bf16 on host; intermediates cast
during psum->sbuf copies). Accumulation is fp32 in PSUM throughout.
"""

import ml_dtypes
import numpy as np

import concourse.bass as bass
from concourse import bacc
import concourse.mybir as mybir
import concourse.tile as tile
from concourse.bass_utils import run_bass_kernel_spmd

B, I, J, D, E = 8, 512, 4096, 1024, 1024
H, DH = 16, 64
JC = J // 128   # 32
IC = I // 128   # 4
DC = D // 128   # 8
EC = E // 128   # 8
F32 = mybir.dt.float32
BF16 = mybir.dt.bfloat16
BF = ml_dtypes.bfloat16

JP_RESIDENT_MAX = 2816  # X^T fully SBUF-resident up to this padded kv length


def _mix(*streams):
    """Proportionally interleave step generators.

    Each stream is (generator, approx_step_count).  Steps are drawn so every
    stream's completed fraction stays balanced — this sets the PE program
    order so K^T-projection matmuls fill the gaps the attention loop leaves
    while it waits on Activation-engine exps."""
    live = [[g, n, 0] for g, n in streams if n > 0]
    while live:
        g, n, done = min(live, key=lambda s: s[2] / s[1])
        try:
            next(g)
            for s in live:
                if s[0] is g:
                    s[2] += 1
        except StopIteration:
            live = [s for s in live if s[0] is not g]


def build_nc(jp=J, ip=I):
    jcp = jp // 128
    nc = bacc.Bacc()

    txtT = nc.dram_tensor("txtT", [D, I], BF16, kind="ExternalInput")
    imT = nc.dram_tensor("imT", [D, jp], BF16, kind="ExternalInput")
    wq = nc.dram_tensor("wq", [D, E], BF16, kind="ExternalInput")
    wkv = nc.dram_tensor("wkv", [D, 2 * E], BF16, kind="ExternalInput")
    wout = nc.dram_tensor("wout", [E, D], BF16, kind="ExternalInput")
    kvmp = nc.dram_tensor("kvmp", [128, jcp], F32, kind="ExternalInput")
    qmp = nc.dram_tensor("qmp", [128, IC], F32, kind="ExternalInput")
    qmrow = nc.dram_tensor("qmrow", [1, I], BF16, kind="ExternalInput")
    omqrow = nc.dram_tensor("omqrow", [1, I], BF16, kind="ExternalInput")
    ymeanb = nc.dram_tensor("ymeanb", [1, D], BF16, kind="ExternalInput")
    boutr = nc.dram_tensor("boutr", [1, D], BF16, kind="ExternalInput")
    y = nc.dram_tensor("y", [I, D], BF16, kind="ExternalOutput")

    jblocks = []
    off = 0
    while off < jp:
        w = 512 if jp - off >= 512 else jp - off
        jblocks.append((off, w))
        off += w

    with tile.TileContext(nc) as tc:
        with (
            tc.tile_pool(name="wpool", bufs=1) as wpool,      # small resident
            tc.tile_pool(name="wslot", bufs=3) as wslot,      # wq/wv/wk live together
            tc.tile_pool(name="big", bufs=1) as big,          # imr, K^T, V, Q^T, O^T
            tc.tile_pool(name="work", bufs=3) as work,        # small working tiles
            tc.tile_pool(name="etp", bufs=4) as etp,
            tc.tile_pool(name="ppool", bufs=3, space="PSUM") as ppool,   # 6 banks
            tc.tile_pool(name="accp", bufs=2, space="PSUM") as accp,     # 2 banks
        ):
            # phase Q's operands go first — every DMA transfer serializes
            # through the DMA engines, so these two gate PE start
            ta = wpool.tile([128, DC, I], BF16)
            nc.sync.dma_start(ta[:], txtT[:].rearrange("(dc p) i -> p dc i", p=128))
            wq_sb = wslot.tile([128, DC, E], BF16, tag="w")
            nc.sync.dma_start(wq_sb[:], wq[:].rearrange("(dc p) e -> p dc e", p=128))

            # ---- resident small inputs ----
            kvm_sb = wpool.tile([128, jcp], F32)
            nc.sync.dma_start(kvm_sb[:], kvmp[:])
            qmp_sb = wpool.tile([128, IC], F32)
            nc.sync.dma_start(qmp_sb[:], qmp[:])
            qmrow_sb = wpool.tile([1, I], BF16)
            nc.sync.dma_start(qmrow_sb[:], qmrow[:])
            omqrow_sb = wpool.tile([1, I], BF16)
            nc.sync.dma_start(omqrow_sb[:], omqrow[:])
            ymeanb_sb = wpool.tile([1, D], BF16)
            nc.sync.dma_start(ymeanb_sb[:], ymeanb[:])
            boutr_sb = wpool.tile([1, D], BF16)
            nc.sync.dma_start(boutr_sb[:], boutr[:])
            ones64 = wpool.tile([1, DH], BF16)
            nc.vector.memset(ones64[:], 1.0)

            # ---- big residents: X^T once from HBM, K^T/V/Q^T/O^T on-chip ----
            # DMA order is the startup critical path: phase Q needs wq+ta
            # first, then V needs wv+imr, then a_chunk(0) needs wk.  imr is
            # loaded block-by-block so phase V's first chunks don't wait for
            # the full 4.5MB transfer (subtile deps track the slices).
            imr = big.tile([128, DC, jp], BF16)
            KT_sb = big.tile([128, EC, jp], BF16)
            QT_sb = big.tile([128, EC, I], BF16)
            OT_sb = big.tile([128, EC, I], BF16)
            nc.vector.memset(OT_sb[:], 0.0)
            V_sb = big.tile([128, jcp, H, DH + 1], BF16)
            nc.vector.tensor_copy(
                V_sb[:, :, :, DH : DH + 1],
                kvm_sb[:, :, None, None].to_broadcast([128, jcp, H, 1]),
            )

            wv_sb = wslot.tile([128, DC, E], BF16, tag="w")
            nc.scalar.dma_start(
                wv_sb[:], wkv[:, E : 2 * E].rearrange("(dc p) e -> p dc e", p=128)
            )
            imTr = imT[:].rearrange("(dc p) j -> p dc j", p=128)
            for off, w in [(o, min(512, jp - o)) for o in range(0, jp, 512)]:
                nc.scalar.dma_start(
                    imr[:, :, off : off + w], imTr[:, :, off : off + w]
                )
            wk_sb = wslot.tile([128, DC, E], BF16, tag="w")
            nc.scalar.dma_start(
                wk_sb[:], wkv[:, 0:E].rearrange("(dc p) e -> p dc e", p=128)
            )

            # ============ Phase Q: Q^T = Wq^T @ T^T, scaled 1/8 ==========
            for ep in range(EC // 2):
                ps = ppool.tile([128, 2, 512], F32, tag="sps", bufs=2)
                for eh in range(2):
                    ec = 2 * ep + eh
                    for dc in range(DC):
                        nc.tensor.matmul(
                            ps[:, eh, 0:ip],
                            wq_sb[:, dc, ec * 128 : (ec + 1) * 128],
                            ta[:, dc, 0:ip],
                            start=(dc == 0),
                            stop=(dc == DC - 1),
                        )
                nc.vector.tensor_scalar_mul(
                    QT_sb[:, 2 * ep : 2 * ep + 2, 0:ip], ps[:, :, 0:ip], 0.125
                )

            # ============ Phase V: V' = (X Wv) * kvm, from resident X^T ====
            def v_phase():
                for jc in range(jcp):
                    ps = ppool.tile([128, 2, 512], F32, tag="sps", bufs=2)
                    for eb in range(2):
                        for dc in range(DC):
                            nc.tensor.matmul(
                                ps[:, eb, :],
                                imr[:, dc, jc * 128 : (jc + 1) * 128],
                                wv_sb[:, dc, eb * 512 : (eb + 1) * 512],
                                start=(dc == 0),
                                stop=(dc == DC - 1),
                            )
                    nc.vector.tensor_scalar_mul(
                        V_sb[:, jc, :, 0:DH],
                        ps[:].rearrange("p b (h dh) -> p (b h) dh", dh=DH),
                        kvm_sb[:, jc : jc + 1],
                    )
                    yield

            # ====== Main loop: K^T chunks pipelined against attention ======
            def a_chunk(ep):
                # K^T rows for ec chunks 2ep, 2ep+1, from resident X^T
                for off, w in jblocks:
                    ps = ppool.tile([128, 2, 512], F32, tag="aps", bufs=1)
                    for eh in range(2):
                        ec = 2 * ep + eh
                        for dc in range(DC):
                            nc.tensor.matmul(
                                ps[:, eh, 0:w],
                                wk_sb[:, dc, ec * 128 : (ec + 1) * 128],
                                imr[:, dc, off : off + w],
                                start=(dc == 0),
                                stop=(dc == DC - 1),
                            )
                        yield
                    nc.vector.tensor_copy(
                        KT_sb[:, 2 * ep : 2 * ep + 2, off : off + w],
                        ps[:, :, 0:w],
                    )

            def c_pair(hp):
                vt = V_sb[:, :, 2 * hp : 2 * hp + 2, :]
                oacc_a = accp.tile([DH + 1, 512], F32, tag="oacc")
                oacc_b = accp.tile([DH + 1, 512], F32, tag="oacc")

                def qk(jc):
                    sps = ppool.tile([128, 2, 512], F32, tag="sps", bufs=2)
                    nc.tensor.matmul(
                        sps[:, 0, 0:ip],
                        KT_sb[0:DH, hp, jc * 128 : (jc + 1) * 128],
                        QT_sb[0:DH, hp, 0:ip],
                        start=True,
                        stop=True,
                    )
                    nc.tensor.matmul(
                        sps[:, 1, 0:ip],
                        KT_sb[DH:128, hp, jc * 128 : (jc + 1) * 128],
                        QT_sb[DH:128, hp, 0:ip],
                        start=True,
                        stop=True,
                    )
                    et = etp.tile([128, 2, ip], BF16, tag="et")
                    nc.scalar.activation(
                        et[:, :, 0:ip],
                        sps[:, :, 0:ip],
                        mybir.ActivationFunctionType.Exp,
                        scale=kvm_sb[:, jc : jc + 1],
                    )
                    return et

                def pv(jc, et):
                    nc.tensor.matmul(
                        oacc_a[:, 0:ip],
                        vt[:, jc, 0, :],
                        et[:, 0, 0:ip],
                        start=(jc == 0),
                        stop=(jc == jcp - 1),
                    )
                    nc.tensor.matmul(
                        oacc_b[:, 0:ip],
                        vt[:, jc, 1, :],
                        et[:, 1, 0:ip],
                        start=(jc == 0),
                        stop=(jc == jcp - 1),
                    )

                # software pipeline: QK^T issued two chunks ahead of PV so PE
                # never waits on the Activation engine's exp
                ets = [qk(0)]
                yield
                ets.append(qk(1))
                yield
                for jc in range(2, jcp):
                    ets.append(qk(jc))
                    pv(jc - 2, ets.pop(0))
                    yield
                pv(jcp - 2, ets.pop(0))
                yield
                pv(jcp - 1, ets.pop(0))

                # denominator reciprocals first (DVE), then the PE broadcasts —
                # with yields between so interleaved a_chunk matmuls cover the
                # DVE latency
                recbs = []
                for hh, oacc in ((0, oacc_a), (1, oacc_b)):
                    rec = work.tile([1, ip], F32, tag="rec")
                    nc.vector.reciprocal(rec[:, 0:ip], oacc[DH : DH + 1, 0:ip])
                    recb = work.tile([1, ip], BF16, tag="recb")
                    nc.vector.tensor_copy(recb[:, 0:ip], rec[:, 0:ip])
                    recbs.append(recb)
                yield
                for hh, oacc in ((0, oacc_a), (1, oacc_b)):
                    bps = ppool.tile([128, 2, 512], F32, tag="sps", bufs=2)
                    nc.tensor.matmul(
                        bps[0:DH, 0, 0:ip],
                        ones64[:],
                        recbs[hh][:, 0:ip],
                        start=True,
                        stop=True,
                    )
                    rb = work.tile([DH, ip], F32, tag="rb")
                    nc.vector.tensor_copy(rb[:, 0:ip], bps[0:DH, 0, 0:ip])
                    nc.vector.tensor_tensor(
                        OT_sb[hh * DH : (hh + 1) * DH, hp, 0:ip],
                        oacc[0:DH, 0:ip],
                        rb[:, 0:ip],
                        mybir.AluOpType.mult,
                    )
                    yield

            # prefetch the output-projection weights into wq's slot (its
            # readers are done after phase Q, so this overlaps the main loop)
            wo_sb = wslot.tile([128, DC, E], BF16, tag="w")
            nc.gpsimd.dma_start(
                wo_sb[:], wout[:].rearrange("(ec p) d -> p ec d", p=128)
            )

            def c_chain(hp0):
                # the two attention pairs for one K^T chunk, back-to-back
                # (sequential so only one pair's PSUM accumulators are live)
                yield from c_pair(hp0)
                yield from c_pair(hp0 + 1)

            n_a = 2 * len(jblocks)        # yields per a_chunk
            n_c = 2 * (jcp + 4)           # yields per c_chain
            _mix((v_phase(), jcp), (a_chunk(0), n_a))
            for ep in range(1, EC // 2):
                _mix((a_chunk(ep), n_a), (c_chain(2 * ep - 2), n_c))
            # drain the last two attention pairs
            _mix((c_chain(EC - 2), n_c))

            # ============ Phase D: Y = O^T.T @ Wout + blend ==============
            ic_order = sorted(range(IC), key=lambda ic: ic * 128 < ip)
            for ic in ic_order:
                has_valid = ic * 128 < ip
                if has_valid:
                    yps = ppool.tile([128, 2, 512], F32, tag="sps", bufs=2)
                    for db in range(2):
                        for ec in range(EC):
                            nc.tensor.matmul(
                                yps[:, db, :],
                                OT_sb[:, ec, ic * 128 : (ic + 1) * 128],
                                wo_sb[:, ec, db * 512 : (db + 1) * 512],
                                start=(ec == 0),
                                stop=(ec == EC - 1),
                            )
                bb_a = accp.tile([128, 512], F32, tag="oacc")
                bb_b = accp.tile([128, 512], F32, tag="oacc")
                for db, bbps in ((0, bb_a), (1, bb_b)):
                    nc.tensor.matmul(
                        bbps[:],
                        omqrow_sb[:, ic * 128 : (ic + 1) * 128],
                        ymeanb_sb[:, db * 512 : (db + 1) * 512],
                        start=True,
                        stop=False,
                    )
                    nc.tensor.matmul(
                        bbps[:],
                        qmrow_sb[:, ic * 128 : (ic + 1) * 128],
                        boutr_sb[:, db * 512 : (db + 1) * 512],
                        start=False,
                        stop=True,
                    )
                y1 = work.tile([128, 2, 512], BF16, tag="y1", bufs=2)
                if has_valid:
                    nc.vector.tensor_scalar_mul(
                        y1[:], yps[:], qmp_sb[:, ic : ic + 1]
                    )
                    nc.vector.tensor_tensor(
                        y1[:, 0, :], bb_a[:], y1[:, 0, :], mybir.AluOpType.add
                    )
                    nc.vector.tensor_tensor(
                        y1[:, 1, :], bb_b[:], y1[:, 1, :], mybir.AluOpType.add
                    )
                else:
                    nc.vector.tensor_copy(y1[:, 0, :], bb_a[:])
                    nc.vector.tensor_copy(y1[:, 1, :], bb_b[:])
                nc.sync.dma_start(
                    y[ic * 128 : (ic + 1) * 128, :],
                    y1[:].rearrange("p b d -> p (b d)"),
                )

    nc.compile()
    return nc


def build_nc_stream(jp=J, ip=I):
    """Streaming fallback for jp > JP_RESIDENT_MAX (X^T doesn't fit SBUF):
    the original 5-pass streaming kernel, V through a DRAM round-trip."""
    jcp = jp // 128
    nc = bacc.Bacc()

    txtT = nc.dram_tensor("txtT", [D, I], BF16, kind="ExternalInput")
    imT = nc.dram_tensor("imT", [D, jp], BF16, kind="ExternalInput")
    wq = nc.dram_tensor("wq", [D, E], BF16, kind="ExternalInput")
    wkv = nc.dram_tensor("wkv", [D, 2 * E], BF16, kind="ExternalInput")
    wout = nc.dram_tensor("wout", [E, D], BF16, kind="ExternalInput")
    kvmp = nc.dram_tensor("kvmp", [128, jcp], F32, kind="ExternalInput")
    qmp = nc.dram_tensor("qmp", [128, IC], F32, kind="ExternalInput")
    qmrow = nc.dram_tensor("qmrow", [1, I], BF16, kind="ExternalInput")
    omqrow = nc.dram_tensor("omqrow", [1, I], BF16, kind="ExternalInput")
    ymeanb = nc.dram_tensor("ymeanb", [1, D], BF16, kind="ExternalInput")
    boutr = nc.dram_tensor("boutr", [1, D], BF16, kind="ExternalInput")
    y = nc.dram_tensor("y", [I, D], F32, kind="ExternalOutput")
    vdr = nc.dram_tensor("vdr", [jcp, 128, H, DH], BF16, kind="Internal")

    imTr = imT[:].rearrange("(dc p) j -> p dc j", p=128)

    with tile.TileContext(nc) as tc:
        with (
            tc.tile_pool(name="wpool", bufs=1) as wpool,
            tc.tile_pool(name="wslot", bufs=1) as wslot,
            tc.tile_pool(name="big", bufs=1) as big,
            tc.tile_pool(name="stream", bufs=2) as stream,
            tc.tile_pool(name="work", bufs=3) as work,
            tc.tile_pool(name="etp", bufs=7) as etp,
            tc.tile_pool(name="ppool", bufs=2, space="PSUM") as ppool,
            tc.tile_pool(name="accp", bufs=3, space="PSUM") as accp,
            tc.tile_pool(name="bcp", bufs=1, space="PSUM") as bcp,
        ):
            kvm_sb = wpool.tile([128, jcp], F32)
            nc.sync.dma_start(kvm_sb[:], kvmp[:])
            qmp_sb = wpool.tile([128, IC], F32)
            nc.sync.dma_start(qmp_sb[:], qmp[:])
            qmrow_sb = wpool.tile([1, I], BF16)
            nc.sync.dma_start(qmrow_sb[:], qmrow[:])
            omqrow_sb = wpool.tile([1, I], BF16)
            nc.sync.dma_start(omqrow_sb[:], omqrow[:])
            ymeanb_sb = wpool.tile([1, D], BF16)
            nc.sync.dma_start(ymeanb_sb[:], ymeanb[:])
            boutr_sb = wpool.tile([1, D], BF16)
            nc.sync.dma_start(boutr_sb[:], boutr[:])
            ones64 = wpool.tile([1, DH], BF16)
            nc.vector.memset(ones64[:], 1.0)

            KT_sb = big.tile([128, EC, jp], BF16)
            QT_sb = big.tile([128, EC, I], BF16)
            OT_sb = big.tile([128, EC, I], BF16)
            nc.vector.memset(OT_sb[:], 0.0)

            wq_sb = wslot.tile([128, DC, E], BF16, tag="w")
            nc.gpsimd.dma_start(wq_sb[:], wq[:].rearrange("(dc p) e -> p dc e", p=128))
            ta = stream.tile([128, DC, I], BF16, tag="im")
            nc.gpsimd.dma_start(ta[:], txtT[:].rearrange("(dc p) i -> p dc i", p=128))
            for ep in range(EC // 2):
                ps = ppool.tile([128, 2, 512], F32, tag="s2")
                for eh in range(2):
                    ec = 2 * ep + eh
                    for dc in range(DC):
                        nc.tensor.matmul(
                            ps[:, eh, 0:ip],
                            wq_sb[:, dc, ec * 128 : (ec + 1) * 128],
                            ta[:, dc, 0:ip],
                            start=(dc == 0),
                            stop=(dc == DC - 1),
                        )
                nc.vector.tensor_scalar_mul(
                    QT_sb[:, 2 * ep : 2 * ep + 2, 0:ip], ps[:, :, 0:ip], 0.125
                )

            wv_sb = wslot.tile([128, DC, E], BF16, tag="w")
            nc.gpsimd.dma_start(
                wv_sb[:], wkv[:, E : 2 * E].rearrange("(dc p) e -> p dc e", p=128)
            )
            wk_sb = wslot.tile([128, DC, E], BF16, tag="w")
            nc.gpsimd.dma_start(
                wk_sb[:], wkv[:, 0:E].rearrange("(dc p) e -> p dc e", p=128)
            )

            jblocks = []
            off = 0
            while off < jp:
                w = 512 if jp - off >= 512 else jp - off
                jblocks.append((off, w))
                off += w

            def b_all():
                for off, w in jblocks:
                    imb = stream.tile([128, DC, 512], BF16, tag="ima")
                    nc.gpsimd.dma_start(
                        imb[:, :, 0:w], imTr[:, :, off : off + w]
                    )
                    for jh in range(w // 128):
                        jc = off // 128 + jh
                        ps = ppool.tile([128, 2, 512], F32, tag="s2")
                        for eb in range(2):
                            for dc in range(DC):
                                nc.tensor.matmul(
                                    ps[:, eb, :],
                                    imb[:, dc, jh * 128 : (jh + 1) * 128],
                                    wv_sb[:, dc, eb * 512 : (eb + 1) * 512],
                                    start=(dc == 0),
                                    stop=(dc == DC - 1),
                                )
                        vtmp = work.tile([128, H, DH], BF16, tag="vtmp")
                        nc.vector.tensor_scalar_mul(
                            vtmp[:],
                            ps[:].rearrange("p b (h dh) -> p (b h) dh", dh=DH),
                            kvm_sb[:, jc : jc + 1],
                        )
                        nc.sync.dma_start(vdr[jc, :, :, :], vtmp[:])

            def a_chunk(ep):
                for off, w in jblocks:
                    ima = stream.tile([128, DC, 512], BF16, tag="ima")
                    nc.gpsimd.dma_start(ima[:, :, 0:w], imTr[:, :, off : off + w])
                    ps = ppool.tile([128, 2, 512], F32, tag="s2")
                    for eh in range(2):
                        ec = 2 * ep + eh
                        for dc in range(DC):
                            nc.tensor.matmul(
                                ps[:, eh, 0:w],
                                wk_sb[:, dc, ec * 128 : (ec + 1) * 128],
                                ima[:, dc, 0:w],
                                start=(dc == 0),
                                stop=(dc == DC - 1),
                            )
                    nc.vector.tensor_copy(
                        KT_sb[:, 2 * ep : 2 * ep + 2, off : off + w],
                        ps[:, :, 0:w],
                    )

            def c_pair(hp):
                vtt = stream.tile([128, jcp, 2, DH + 1], BF16, tag="vt")
                nc.vector.tensor_copy(
                    vtt[:, :, 0, DH : DH + 1], kvm_sb[:, :, None]
                )
                nc.vector.tensor_copy(
                    vtt[:, :, 1, DH : DH + 1], kvm_sb[:, :, None]
                )
                for hh in range(2):
                    nc.sync.dma_start(
                        vtt[:, :, hh, 0:DH],
                        vdr[:, :, 2 * hp + hh, :].rearrange("jc p dh -> p jc dh"),
                    )
                vt = vtt
                oacc_a = accp.tile([DH + 1, 512], F32, tag="oacc")
                oacc_b = accp.tile([DH + 1, 512], F32, tag="oacc")
                for jc in range(jcp):
                    sps = ppool.tile([128, 2, 512], F32, tag="s2")
                    nc.tensor.matmul(
                        sps[:, 0, 0:ip],
                        KT_sb[0:DH, hp, jc * 128 : (jc + 1) * 128],
                        QT_sb[0:DH, hp, 0:ip],
                        start=True,
                        stop=True,
                    )
                    nc.tensor.matmul(
                        sps[:, 1, 0:ip],
                        KT_sb[DH:128, hp, jc * 128 : (jc + 1) * 128],
                        QT_sb[DH:128, hp, 0:ip],
                        start=True,
                        stop=True,
                    )
                    et = etp.tile([128, 2, ip], BF16, tag="et")
                    nc.scalar.activation(
                        et[:, :, 0:ip],
                        sps[:, :, 0:ip],
                        mybir.ActivationFunctionType.Exp,
                        scale=kvm_sb[:, jc : jc + 1],
                    )
                    nc.tensor.matmul(
                        oacc_a[:, 0:ip],
                        vt[:, jc, 0, :],
                        et[:, 0, 0:ip],
                        start=(jc == 0),
                        stop=(jc == jcp - 1),
                    )
                    nc.tensor.matmul(
                        oacc_b[:, 0:ip],
                        vt[:, jc, 1, :],
                        et[:, 1, 0:ip],
                        start=(jc == 0),
                        stop=(jc == jcp - 1),
                    )
                for hh, oacc in ((0, oacc_a), (1, oacc_b)):
                    rec = work.tile([1, ip], F32, tag="rec")
                    nc.vector.reciprocal(rec[:, 0:ip], oacc[DH : DH + 1, 0:ip])
                    recb = work.tile([1, ip], BF16, tag="recb")
                    nc.vector.tensor_copy(recb[:, 0:ip], rec[:, 0:ip])
                    bps = bcp.tile([DH, 512], F32, tag="bc")
                    nc.tensor.matmul(
                        bps[:, 0:ip],
                        ones64[:],
                        recb[:, 0:ip],
                        start=True,
                        stop=True,
                    )
                    rb = work.tile([DH, ip], F32, tag="rb")
                    nc.vector.tensor_copy(rb[:, 0:ip], bps[:, 0:ip])
                    nc.vector.tensor_tensor(
                        OT_sb[hh * DH : (hh + 1) * DH, hp, 0:ip],
                        oacc[0:DH, 0:ip],
                        rb[:, 0:ip],
                        mybir.AluOpType.mult,
                    )

            b_all()
            for ep in range(EC // 2):
                a_chunk(ep)
                c_pair(2 * ep)
                c_pair(2 * ep + 1)

            wo_sb = wslot.tile([128, DC, E], BF16, tag="w")
            nc.gpsimd.dma_start(
                wo_sb[:], wout[:].rearrange("(ec p) d -> p ec d", p=128)
            )
            for ic in range(IC):
                has_valid = ic * 128 < ip
                if has_valid:
                    yps = ppool.tile([128, 2, 512], F32, tag="s2")
                    for db in range(2):
                        for ec in range(EC):
                            nc.tensor.matmul(
                                yps[:, db, :],
                                OT_sb[:, ec, ic * 128 : (ic + 1) * 128],
                                wo_sb[:, ec, db * 512 : (db + 1) * 512],
                                start=(ec == 0),
                                stop=(ec == EC - 1),
                            )
                bb_a = accp.tile([128, 512], F32, tag="oacc")
                bb_b = accp.tile([128, 512], F32, tag="oacc")
                for db, bbps in ((0, bb_a), (1, bb_b)):
                    nc.tensor.matmul(
                        bbps[:],
                        omqrow_sb[:, ic * 128 : (ic + 1) * 128],
                        ymeanb_sb[:, db * 512 : (db + 1) * 512],
                        start=True,
                        stop=False,
                    )
                    nc.tensor.matmul(
                        bbps[:],
                        qmrow_sb[:, ic * 128 : (ic + 1) * 128],
                        boutr_sb[:, db * 512 : (db + 1) * 512],
                        start=False,
                        stop=True,
                    )
                y1 = work.tile([128, 2, 512], F32, tag="y1")
                if has_valid:
                    nc.vector.tensor_scalar_mul(
                        y1[:], yps[:], qmp_sb[:, ic : ic + 1]
                    )
                    nc.vector.tensor_tensor(
                        y1[:, 0, :], bb_a[:], y1[:, 0, :], mybir.AluOpType.add
                    )
                    nc.vector.tensor_tensor(
                        y1[:, 1, :], bb_b[:], y1[:, 1, :], mybir.AluOpType.add
                    )
                else:
                    nc.vector.tensor_copy(y1[:, 0, :], bb_a[:])
                    nc.vector.tensor_copy(y1[:, 1, :], bb_b[:])
                nc.sync.dma_start(
                    y[ic * 128 : (ic + 1) * 128, :],
                    y1[:].rearrange("p b d -> p (b d)"),
                )

    nc.compile()
    return nc


_NC_CACHE = {}


def _get_nc(jp=J, ip=I):
    key = (jp, ip)
    if key not in _NC_CACHE:
        build = build_nc if jp <= JP_RESIDENT_MAX else build_nc_stream
        _NC_CACHE[key] = build(jp, ip)
    return _NC_CACHE[key]


def prep_inputs(txt, image, kv_mask, q_mask, Wq, Wkv, Wout, bout):
    f32 = np.float32
    Wq = np.asarray(Wq, dtype=f32)
    Wkv = np.asarray(Wkv, dtype=f32)
    Wout = np.asarray(Wout, dtype=f32)
    bout = np.asarray(bout, dtype=f32)
    wq_b = Wq.astype(BF)
    wkv_b = Wkv.astype(BF)
    wout_b = Wout.astype(BF)
    kvc = kv_mask.sum(axis=1).max()
    qc = q_mask.sum(axis=1).max()
    jp = max(512, int(-(-kvc // 128)) * 128)
    ip = max(256, int(-(-qc // 16)) * 16)
    jcp = jp // 128
    in_maps = []
    perms = []
    for b in range(B):
        kvm = kv_mask[b].astype(bool)
        qm = q_mask[b].astype(bool)
        nkv = int(kvm.sum())
        # compact image columns to valid kv positions, zero-pad to jp
        imTc = np.zeros((D, jp), dtype=BF)
        imTc[:, :nkv] = np.ascontiguousarray(image[b][kvm].T).astype(BF)
        kvmp = np.zeros(jp, dtype=f32)
        kvmp[:nkv] = 1.0
        # permute txt rows valid-first
        perm = np.argsort(~qm, kind="stable")
        perms.append(perm)
        qmperm = qm[perm].astype(f32)
        xmean = image[b].astype(f32).mean(axis=0)
        vmean = xmean @ Wkv[:, E:]
        ymb = vmean @ Wout + bout
        in_maps.append(
            {
                "txtT": np.ascontiguousarray(txt[b][perm].T).astype(BF),
                "imT": imTc,
                "wq": wq_b,
                "wkv": wkv_b,
                "wout": wout_b,
                "kvmp": np.ascontiguousarray(kvmp.reshape(jcp, 128).T),
                "qmp": np.ascontiguousarray(qmperm.reshape(IC, 128).T),
                "qmrow": qmperm[None, :].astype(BF),
                "omqrow": (1.0 - qmperm)[None, :].astype(BF),
                "ymeanb": ymb[None, :].astype(BF),
                "boutr": bout[None, :].astype(BF),
            }
        )
    return in_maps, perms, jp, ip


def run(inputs, trace=False):
    in_maps, perms, jp, ip = prep_inputs(**inputs)
    nc = _get_nc(jp, ip)
    res = run_bass_kernel_spmd(
        nc, in_maps, core_ids=list(range(B)), trace=trace,
        **({"trace_cores": [0]} if trace else {}),
    )
    out = np.empty((B, I, D), dtype=np.float32)
    for b in range(B):
        out[b][perms[b]] = res.results[b]["y"]
    return out, res


def kernel(**inputs):
    out, _ = run(inputs, trace=False)
    return out


# revision 15
# speedup vs baseline: 1.1018x; 1.1018x over previous
"""Cross-attention (txt queries -> image kv) Trainium2 Bass kernel.

Sharding: data-parallel over batch — B=8 batches, one NeuronCore each.
Host-side prep: image columns are COMPACTED to valid kv positions (padded to
jp, a multiple of 256) and txt rows PERMUTED valid-first (attention runs on
the first ip columns only; outputs un-permuted on host). Invalid-q rows are
reconstructed exactly via the ymeanb blend (uniform attention over all kv).
Per core (batch b):
  Q^T = Wq^T T^T / 8          [e, i]   bf16 SBUF-resident
  K^T = Wk^T X^T              [e, j]   bf16 SBUF-resident
  V'  = (X Wv) * kvm_j        [j, e]   bf16 SBUF-resident; per head tiles
                                       [j, h, 65] whose col 64 holds kvm_j
                                       (so PV accumulates both numerator and
                                       softmax denominator with the kv mask
                                       applied exactly)
  S^T_h = K_h Q_h^T           [j, i]   psum, two heads per [128,2,512] tile
  P^T = exp(S^T * kvm_j)      (ACT scale=kvm; masked/padded rows give exp(0)=1
                               but are zeroed by V' — no separate mask op)
  O^T_aug_h = [V'_h | kvm] ^T P^T -> [65, i] psum accum over jchunks
  O^T = O^T_aug[0:64] * recip(denom) broadcast via PE ones-outer-product
  Y = O^T.T Wout; blend: y = qm_i*Y + (1-qm_i)*ymeanb + qm_i*bout
    (q_mask=False rows = uniform attention over all kv -> host-computed
     ymeanb = (mean_j X) @ Wv @ Wout + bout)

Fast path (jp <= 2816): X^T kept fully SBUF-resident — read from HBM exactly
once (the old streaming path re-read it 5x).  K^T chunk computation (PE) is
software-pipelined against the previous chunk's attention (QK^T/exp/PV), so
the Activation engine's exp work hides behind PE matmuls, and within the
attention inner loop QK^T(jc+1) is issued ahead of PV(jc) so PE never waits
on the exp of the current chunk.

All PE matmuls in bf16 (inputs pre-cast to bf16 on host; intermediates cast
during psum->sbuf copies). Accumulation is fp32 in PSUM throughout.
"""

import ml_dtypes
import numpy as np

import concourse.bass as bass
from concourse import bacc
import concourse.mybir as mybir
import concourse.tile as tile
from concourse.bass_utils import run_bass_kernel_spmd

B, I, J, D, E = 8, 512, 4096, 1024, 1024
H, DH = 16, 64
JC = J // 128   # 32
IC = I // 128   # 4
DC = D // 128   # 8
EC = E // 128   # 8
F32 = mybir.dt.float32
BF16 = mybir.dt.bfloat16
BF = ml_dtypes.bfloat16

JP_RESIDENT_MAX = 2816  # X^T fully SBUF-resident up to this padded kv length


def _mix(*streams):
    """Proportionally interleave step generators.

    Each stream is (generator, approx_step_count).  Steps are drawn so every
    stream's completed fraction stays balanced — this sets the PE program
    order so K^T-projection matmuls fill the gaps the attention loop leaves
    while it waits on Activation-engine exps."""
    live = [[g, n, 0] for g, n in streams if n > 0]
    while live:
        g, n, done = min(live, key=lambda s: s[2] / s[1])
        try:
            next(g)
            for s in live:
                if s[0] is g:
                    s[2] += 1
        except StopIteration:
            live = [s for s in live if s[0] is not g]


def build_nc(jp=J, ip=I):
    jcp = jp // 128
    nc = bacc.Bacc()

    txtT = nc.dram_tensor("txtT", [D, I], BF16, kind="ExternalInput")
    imT = nc.dram_tensor("imT", [D, jp], BF16, kind="ExternalInput")
    wq = nc.dram_tensor("wq", [D, E], BF16, kind="ExternalInput")
    wkv = nc.dram_tensor("wkv", [D, 2 * E], BF16, kind="ExternalInput")
    wout = nc.dram_tensor("wout", [E, D], BF16, kind="ExternalInput")
    kvmp = nc.dram_tensor("kvmp", [128, jcp], F32, kind="ExternalInput")
    qmp = nc.dram_tensor("qmp", [128, IC], F32, kind="ExternalInput")
    qmrow = nc.dram_tensor("qmrow", [1, I], BF16, kind="ExternalInput")
    omqrow = nc.dram_tensor("omqrow", [1, I], BF16, kind="ExternalInput")
    ymeanb = nc.dram_tensor("ymeanb", [1, D], BF16, kind="ExternalInput")
    boutr = nc.dram_tensor("boutr", [1, D], BF16, kind="ExternalInput")
    y = nc.dram_tensor("y", [I, D], BF16, kind="ExternalOutput")

    jblocks = []
    off = 0
    while off < jp:
        w = 512 if jp - off >= 512 else jp - off
        jblocks.append((off, w))
        off += w

    with tile.TileContext(nc) as tc:
        with (
            tc.tile_pool(name="wpool", bufs=1) as wpool,      # small resident
            tc.tile_pool(name="wslot", bufs=3) as wslot,      # wq/wv/wk live together
            tc.tile_pool(name="big", bufs=1) as big,          # imr, K^T, V, Q^T, O^T
            tc.tile_pool(name="work", bufs=3) as work,        # small working tiles
            tc.tile_pool(name="etp", bufs=4) as etp,
            tc.tile_pool(name="ppool", bufs=3, space="PSUM") as ppool,   # 6 banks
            tc.tile_pool(name="accp", bufs=2, space="PSUM") as accp,     # 2 banks
        ):
            # phase Q's operands go first — every DMA transfer serializes
            # through the DMA engines, so these two gate PE start
            ta = wpool.tile([128, DC, I], BF16)
            nc.sync.dma_start(ta[:], txtT[:].rearrange("(dc p) i -> p dc i", p=128))
            wq_sb = wslot.tile([128, DC, E], BF16, tag="w")
            nc.scalar.dma_start(wq_sb[:], wq[:].rearrange("(dc p) e -> p dc e", p=128))

            # ---- resident small inputs ----
            kvm_sb = wpool.tile([128, jcp], F32)
            nc.sync.dma_start(kvm_sb[:], kvmp[:])
            qmp_sb = wpool.tile([128, IC], F32)
            nc.sync.dma_start(qmp_sb[:], qmp[:])
            qmrow_sb = wpool.tile([1, I], BF16)
            nc.sync.dma_start(qmrow_sb[:], qmrow[:])
            omqrow_sb = wpool.tile([1, I], BF16)
            nc.sync.dma_start(omqrow_sb[:], omqrow[:])
            ymeanb_sb = wpool.tile([1, D], BF16)
            nc.sync.dma_start(ymeanb_sb[:], ymeanb[:])
            boutr_sb = wpool.tile([1, D], BF16)
            nc.sync.dma_start(boutr_sb[:], boutr[:])
            ones64 = wpool.tile([1, DH], BF16)
            nc.vector.memset(ones64[:], 1.0)

            # ---- big residents: X^T once from HBM, K^T/V/Q^T/O^T on-chip ----
            # DMA order is the startup critical path: phase Q needs wq+ta
            # first, then V needs wv+imr, then a_chunk(0) needs wk.  imr is
            # loaded block-by-block so phase V's first chunks don't wait for
            # the full 4.5MB transfer (subtile deps track the slices).
            imr = big.tile([128, DC, jp], BF16)
            KT_sb = big.tile([128, EC, jp], BF16)
            QT_sb = big.tile([128, EC, I], BF16)
            OT_sb = big.tile([128, EC, I], BF16)
            nc.vector.memset(OT_sb[:], 0.0)
            V_sb = big.tile([128, jcp, H, DH + 1], BF16)
            nc.vector.tensor_copy(
                V_sb[:, :, :, DH : DH + 1],
                kvm_sb[:, :, None, None].to_broadcast([128, jcp, H, 1]),
            )

            wv_sb = wslot.tile([128, DC, E], BF16, tag="w")
            nc.scalar.dma_start(
                wv_sb[:], wkv[:, E : 2 * E].rearrange("(dc p) e -> p dc e", p=128)
            )
            imTr = imT[:].rearrange("(dc p) j -> p dc j", p=128)
            for off, w in [(o, min(512, jp - o)) for o in range(0, jp, 512)]:
                nc.scalar.dma_start(
                    imr[:, :, off : off + w], imTr[:, :, off : off + w]
                )
            wk_sb = wslot.tile([128, DC, E], BF16, tag="w")
            nc.scalar.dma_start(
                wk_sb[:], wkv[:, 0:E].rearrange("(dc p) e -> p dc e", p=128)
            )

            # ============ Phase Q: Q^T = Wq^T @ T^T, scaled 1/8 ==========
            for ep in range(EC // 2):
                ps = ppool.tile([128, 2, 512], F32, tag="sps", bufs=2)
                for eh in range(2):
                    ec = 2 * ep + eh
                    for dc in range(DC):
                        nc.tensor.matmul(
                            ps[:, eh, 0:ip],
                            wq_sb[:, dc, ec * 128 : (ec + 1) * 128],
                            ta[:, dc, 0:ip],
                            start=(dc == 0),
                            stop=(dc == DC - 1),
                        )
                nc.vector.tensor_scalar_mul(
                    QT_sb[:, 2 * ep : 2 * ep + 2, 0:ip], ps[:, :, 0:ip], 0.125
                )

            # ============ Phase V: V' = (X Wv) * kvm, from resident X^T ====
            def v_phase():
                for jc in range(jcp):
                    ps = ppool.tile([128, 2, 512], F32, tag="sps", bufs=2)
                    for eb in range(2):
                        for dc in range(DC):
                            nc.tensor.matmul(
                                ps[:, eb, :],
                                imr[:, dc, jc * 128 : (jc + 1) * 128],
                                wv_sb[:, dc, eb * 512 : (eb + 1) * 512],
                                start=(dc == 0),
                                stop=(dc == DC - 1),
                            )
                    nc.vector.tensor_scalar_mul(
                        V_sb[:, jc, :, 0:DH],
                        ps[:].rearrange("p b (h dh) -> p (b h) dh", dh=DH),
                        kvm_sb[:, jc : jc + 1],
                    )
                    yield

            # ====== Main loop: K^T chunks pipelined against attention ======
            def a_chunk(ep):
                # K^T rows for ec chunks 2ep, 2ep+1, from resident X^T
                for off, w in jblocks:
                    ps = ppool.tile([128, 2, 512], F32, tag="aps", bufs=1)
                    for eh in range(2):
                        ec = 2 * ep + eh
                        for dc in range(DC):
                            nc.tensor.matmul(
                                ps[:, eh, 0:w],
                                wk_sb[:, dc, ec * 128 : (ec + 1) * 128],
                                imr[:, dc, off : off + w],
                                start=(dc == 0),
                                stop=(dc == DC - 1),
                            )
                        yield
                    nc.vector.tensor_copy(
                        KT_sb[:, 2 * ep : 2 * ep + 2, off : off + w],
                        ps[:, :, 0:w],
                    )

            def c_pair(hp):
                vt = V_sb[:, :, 2 * hp : 2 * hp + 2, :]
                oacc_a = accp.tile([DH + 1, 512], F32, tag="oacc")
                oacc_b = accp.tile([DH + 1, 512], F32, tag="oacc")

                def qk(jc):
                    sps = ppool.tile([128, 2, 512], F32, tag="sps", bufs=2)
                    nc.tensor.matmul(
                        sps[:, 0, 0:ip],
                        KT_sb[0:DH, hp, jc * 128 : (jc + 1) * 128],
                        QT_sb[0:DH, hp, 0:ip],
                        start=True,
                        stop=True,
                    )
                    nc.tensor.matmul(
                        sps[:, 1, 0:ip],
                        KT_sb[DH:128, hp, jc * 128 : (jc + 1) * 128],
                        QT_sb[DH:128, hp, 0:ip],
                        start=True,
                        stop=True,
                    )
                    et = etp.tile([128, 2, ip], BF16, tag="et")
                    nc.scalar.activation(
                        et[:, :, 0:ip],
                        sps[:, :, 0:ip],
                        mybir.ActivationFunctionType.Exp,
                        scale=kvm_sb[:, jc : jc + 1],
                    )
                    return et

                def pv(jc, et):
                    nc.tensor.matmul(
                        oacc_a[:, 0:ip],
                        vt[:, jc, 0, :],
                        et[:, 0, 0:ip],
                        start=(jc == 0),
                        stop=(jc == jcp - 1),
                    )
                    nc.tensor.matmul(
                        oacc_b[:, 0:ip],
                        vt[:, jc, 1, :],
                        et[:, 1, 0:ip],
                        start=(jc == 0),
                        stop=(jc == jcp - 1),
                    )

                # software pipeline: QK^T issued two chunks ahead of PV so PE
                # never waits on the Activation engine's exp
                ets = [qk(0)]
                yield
                ets.append(qk(1))
                yield
                for jc in range(2, jcp):
                    ets.append(qk(jc))
                    pv(jc - 2, ets.pop(0))
                    yield
                pv(jcp - 2, ets.pop(0))
                yield
                pv(jcp - 1, ets.pop(0))

                # denominator reciprocals first (DVE), then the PE broadcasts —
                # with yields between so interleaved a_chunk matmuls cover the
                # DVE latency
                recbs = []
                for hh, oacc in ((0, oacc_a), (1, oacc_b)):
                    rec = work.tile([1, ip], F32, tag="rec")
                    nc.vector.reciprocal(rec[:, 0:ip], oacc[DH : DH + 1, 0:ip])
                    recb = work.tile([1, ip], BF16, tag="recb")
                    nc.vector.tensor_copy(recb[:, 0:ip], rec[:, 0:ip])
                    recbs.append(recb)
                yield
                for hh, oacc in ((0, oacc_a), (1, oacc_b)):
                    bps = ppool.tile([128, 2, 512], F32, tag="sps", bufs=2)
                    nc.tensor.matmul(
                        bps[0:DH, 0, 0:ip],
                        ones64[:],
                        recbs[hh][:, 0:ip],
                        start=True,
                        stop=True,
                    )
                    rb = work.tile([DH, ip], F32, tag="rb")
                    nc.vector.tensor_copy(rb[:, 0:ip], bps[0:DH, 0, 0:ip])
                    nc.vector.tensor_tensor(
                        OT_sb[hh * DH : (hh + 1) * DH, hp, 0:ip],
                        oacc[0:DH, 0:ip],
                        rb[:, 0:ip],
                        mybir.AluOpType.mult,
                    )
                    yield

            # prefetch the output-projection weights into wq's slot (its
            # readers are done after phase Q, so this overlaps the main loop)
            wo_sb = wslot.tile([128, DC, E], BF16, tag="w")
            nc.gpsimd.dma_start(
                wo_sb[:], wout[:].rearrange("(ec p) d -> p ec d", p=128)
            )

            def c_chain(hp0):
                # the two attention pairs for one K^T chunk, back-to-back
                # (sequential so only one pair's PSUM accumulators are live)
                yield from c_pair(hp0)
                yield from c_pair(hp0 + 1)

            n_a = 2 * len(jblocks)        # yields per a_chunk
            n_c = 2 * (jcp + 4)           # yields per c_chain
            _mix((v_phase(), jcp), (a_chunk(0), n_a))
            for ep in range(1, EC // 2):
                _mix((a_chunk(ep), n_a), (c_chain(2 * ep - 2), n_c))
            # drain the last two attention pairs
            _mix((c_chain(EC - 2), n_c))

            # ============ Phase D: Y = O^T.T @ Wout + blend ==============
            ic_order = sorted(range(IC), key=lambda ic: ic * 128 < ip)
            for ic in ic_order:
                has_valid = ic * 128 < ip
                if has_valid:
                    yps = ppool.tile([128, 2, 512], F32, tag="sps", bufs=2)
                    for db in range(2):
                        for ec in range(EC):
                            nc.tensor.matmul(
                                yps[:, db, :],
                                OT_sb[:, ec, ic * 128 : (ic + 1) * 128],
                                wo_sb[:, ec, db * 512 : (db + 1) * 512],
                                start=(ec == 0),
                                stop=(ec == EC - 1),
                            )
                bb_a = accp.tile([128, 512], F32, tag="oacc")
                bb_b = accp.tile([128, 512], F32, tag="oacc")
                for db, bbps in ((0, bb_a), (1, bb_b)):
                    nc.tensor.matmul(
                        bbps[:],
                        omqrow_sb[:, ic * 128 : (ic + 1) * 128],
                        ymeanb_sb[:, db * 512 : (db + 1) * 512],
                        start=True,
                        stop=False,
                    )
                    nc.tensor.matmul(
                        bbps[:],
                        qmrow_sb[:, ic * 128 : (ic + 1) * 128],
                        boutr_sb[:, db * 512 : (db + 1) * 512],
                        start=False,
                        stop=True,
                    )
                # per-half blend + DMA so the first half's store overlaps
                # the second half's vector work
                for db, bb in ((0, bb_a), (1, bb_b)):
                    y1 = work.tile([128, 512], BF16, tag="y1", bufs=3)
                    if has_valid:
                        nc.vector.tensor_scalar_mul(
                            y1[:], yps[:, db, :], qmp_sb[:, ic : ic + 1]
                        )
                        nc.vector.tensor_tensor(
                            y1[:], bb[:], y1[:], mybir.AluOpType.add
                        )
                    else:
                        nc.vector.tensor_copy(y1[:], bb[:])
                    nc.sync.dma_start(
                        y[ic * 128 : (ic + 1) * 128, db * 512 : (db + 1) * 512],
                        y1[:],
                    )

    nc.compile()
    return nc


def build_nc_stream(jp=J, ip=I):
    """Streaming fallback for jp > JP_RESIDENT_MAX (X^T doesn't fit SBUF):
    the original 5-pass streaming kernel, V through a DRAM round-trip."""
    jcp = jp // 128
    nc = bacc.Bacc()

    txtT = nc.dram_tensor("txtT", [D, I], BF16, kind="ExternalInput")
    imT = nc.dram_tensor("imT", [D, jp], BF16, kind="ExternalInput")
    wq = nc.dram_tensor("wq", [D, E], BF16, kind="ExternalInput")
    wkv = nc.dram_tensor("wkv", [D, 2 * E], BF16, kind="ExternalInput")
    wout = nc.dram_tensor("wout", [E, D], BF16, kind="ExternalInput")
    kvmp = nc.dram_tensor("kvmp", [128, jcp], F32, kind="ExternalInput")
    qmp = nc.dram_tensor("qmp", [128, IC], F32, kind="ExternalInput")
    qmrow = nc.dram_tensor("qmrow", [1, I], BF16, kind="ExternalInput")
    omqrow = nc.dram_tensor("omqrow", [1, I], BF16, kind="ExternalInput")
    ymeanb = nc.dram_tensor("ymeanb", [1, D], BF16, kind="ExternalInput")
    boutr = nc.dram_tensor("boutr", [1, D], BF16, kind="ExternalInput")
    y = nc.dram_tensor("y", [I, D], F32, kind="ExternalOutput")
    vdr = nc.dram_tensor("vdr", [jcp, 128, H, DH], BF16, kind="Internal")

    imTr = imT[:].rearrange("(dc p) j -> p dc j", p=128)

    with tile.TileContext(nc) as tc:
        with (
            tc.tile_pool(name="wpool", bufs=1) as wpool,
            tc.tile_pool(name="wslot", bufs=1) as wslot,
            tc.tile_pool(name="big", bufs=1) as big,
            tc.tile_pool(name="stream", bufs=2) as stream,
            tc.tile_pool(name="work", bufs=3) as work,
            tc.tile_pool(name="etp", bufs=7) as etp,
            tc.tile_pool(name="ppool", bufs=2, space="PSUM") as ppool,
            tc.tile_pool(name="accp", bufs=3, space="PSUM") as accp,
            tc.tile_pool(name="bcp", bufs=1, space="PSUM") as bcp,
        ):
            kvm_sb = wpool.tile([128, jcp], F32)
            nc.sync.dma_start(kvm_sb[:], kvmp[:])
            qmp_sb = wpool.tile([128, IC], F32)
            nc.sync.dma_start(qmp_sb[:], qmp[:])
            qmrow_sb = wpool.tile([1, I], BF16)
            nc.sync.dma_start(qmrow_sb[:], qmrow[:])
            omqrow_sb = wpool.tile([1, I], BF16)
            nc.sync.dma_start(omqrow_sb[:], omqrow[:])
            ymeanb_sb = wpool.tile([1, D], BF16)
            nc.sync.dma_start(ymeanb_sb[:], ymeanb[:])
            boutr_sb = wpool.tile([1, D], BF16)
            nc.sync.dma_start(boutr_sb[:], boutr[:])
            ones64 = wpool.tile([1, DH], BF16)
            nc.vector.memset(ones64[:], 1.0)

            KT_sb = big.tile([128, EC, jp], BF16)
            QT_sb = big.tile([128, EC, I], BF16)
            OT_sb = big.tile([128, EC, I], BF16)
            nc.vector.memset(OT_sb[:], 0.0)

            wq_sb = wslot.tile([128, DC, E], BF16, tag="w")
            nc.gpsimd.dma_start(wq_sb[:], wq[:].rearrange("(dc p) e -> p dc e", p=128))
            ta = stream.tile([128, DC, I], BF16, tag="im")
            nc.gpsimd.dma_start(ta[:], txtT[:].rearrange("(dc p) i -> p dc i", p=128))
            for ep in range(EC // 2):
                ps = ppool.tile([128, 2, 512], F32, tag="s2")
                for eh in range(2):
                    ec = 2 * ep + eh
                    for dc in range(DC):
                        nc.tensor.matmul(
                            ps[:, eh, 0:ip],
                            wq_sb[:, dc, ec * 128 : (ec + 1) * 128],
                            ta[:, dc, 0:ip],
                            start=(dc == 0),
                            stop=(dc == DC - 1),
                        )
                nc.vector.tensor_scalar_mul(
                    QT_sb[:, 2 * ep : 2 * ep + 2, 0:ip], ps[:, :, 0:ip], 0.125
                )

            wv_sb = wslot.tile([128, DC, E], BF16, tag="w")
            nc.gpsimd.dma_start(
                wv_sb[:], wkv[:, E : 2 * E].rearrange("(dc p) e -> p dc e", p=128)
            )
            wk_sb = wslot.tile([128, DC, E], BF16, tag="w")
            nc.gpsimd.dma_start(
                wk_sb[:], wkv[:, 0:E].rearrange("(dc p) e -> p dc e", p=128)
            )

            jblocks = []
            off = 0
            while off < jp:
                w = 512 if jp - off >= 512 else jp - off
                jblocks.append((off, w))
                off += w

            def b_all():
                for off, w in jblocks:
                    imb = stream.tile([128, DC, 512], BF16, tag="ima")
                    nc.gpsimd.dma_start(
                        imb[:, :, 0:w], imTr[:, :, off : off + w]
                    )
                    for jh in range(w // 128):
                        jc = off // 128 + jh
                        ps = ppool.tile([128, 2, 512], F32, tag="s2")
                        for eb in range(2):
                            for dc in range(DC):
                                nc.tensor.matmul(
                                    ps[:, eb, :],
                                    imb[:, dc, jh * 128 : (jh + 1) * 128],
                                    wv_sb[:, dc, eb * 512 : (eb + 1) * 512],
                                    start=(dc == 0),
                                    stop=(dc == DC - 1),
                                )
                        vtmp = work.tile([128, H, DH], BF16, tag="vtmp")
                        nc.vector.tensor_scalar_mul(
                            vtmp[:],
                            ps[:].rearrange("p b (h dh) -> p (b h) dh", dh=DH),
                            kvm_sb[:, jc : jc + 1],
                        )
                        nc.sync.dma_start(vdr[jc, :, :, :], vtmp[:])

            def a_chunk(ep):
                for off, w in jblocks:
                    ima = stream.tile([128, DC, 512], BF16, tag="ima")
                    nc.gpsimd.dma_start(ima[:, :, 0:w], imTr[:, :, off : off + w])
                    ps = ppool.tile([128, 2, 512], F32, tag="s2")
                    for eh in range(2):
                        ec = 2 * ep + eh
                        for dc in range(DC):
                            nc.tensor.matmul(
                                ps[:, eh, 0:w],
                                wk_sb[:, dc, ec * 128 : (ec + 1) * 128],
                                ima[:, dc, 0:w],
                                start=(dc == 0),
                                stop=(dc == DC - 1),
                            )
                    nc.vector.tensor_copy(
                        KT_sb[:, 2 * ep : 2 * ep + 2, off : off + w],
                        ps[:, :, 0:w],
                    )

            def c_pair(hp):
                vtt = stream.tile([128, jcp, 2, DH + 1], BF16, tag="vt")
                nc.vector.tensor_copy(
                    vtt[:, :, 0, DH : DH + 1], kvm_sb[:, :, None]
                )
                nc.vector.tensor_copy(
                    vtt[:, :, 1, DH : DH + 1], kvm_sb[:, :, None]
                )
                for hh in range(2):
                    nc.sync.dma_start(
                        vtt[:, :, hh, 0:DH],
                        vdr[:, :, 2 * hp + hh, :].rearrange("jc p dh -> p jc dh"),
                    )
                vt = vtt
                oacc_a = accp.tile([DH + 1, 512], F32, tag="oacc")
                oacc_b = accp.tile([DH + 1, 512], F32, tag="oacc")
                for jc in range(jcp):
                    sps = ppool.tile([128, 2, 512], F32, tag="s2")
                    nc.tensor.matmul(
                        sps[:, 0, 0:ip],
                        KT_sb[0:DH, hp, jc * 128 : (jc + 1) * 128],
                        QT_sb[0:DH, hp, 0:ip],
                        start=True,
                        stop=True,
                    )
                    nc.tensor.matmul(
                        sps[:, 1, 0:ip],
                        KT_sb[DH:128, hp, jc * 128 : (jc + 1) * 128],
                        QT_sb[DH:128, hp, 0:ip],
                        start=True,
                        stop=True,
                    )
                    et = etp.tile([128, 2, ip], BF16, tag="et")
                    nc.scalar.activation(
                        et[:, :, 0:ip],
                        sps[:, :, 0:ip],
                        mybir.ActivationFunctionType.Exp,
                        scale=kvm_sb[:, jc : jc + 1],
                    )
                    nc.tensor.matmul(
                        oacc_a[:, 0:ip],
                        vt[:, jc, 0, :],
                        et[:, 0, 0:ip],
                        start=(jc == 0),
                        stop=(jc == jcp - 1),
                    )
                    nc.tensor.matmul(
                        oacc_b[:, 0:ip],
                        vt[:, jc, 1, :],
                        et[:, 1, 0:ip],
                        start=(jc == 0),
                        stop=(jc == jcp - 1),
                    )
                for hh, oacc in ((0, oacc_a), (1, oacc_b)):
                    rec = work.tile([1, ip], F32, tag="rec")
                    nc.vector.reciprocal(rec[:, 0:ip], oacc[DH : DH + 1, 0:ip])
                    recb = work.tile([1, ip], BF16, tag="recb")
                    nc.vector.tensor_copy(recb[:, 0:ip], rec[:, 0:ip])
                    bps = bcp.tile([DH, 512], F32, tag="bc")
                    nc.tensor.matmul(
                        bps[:, 0:ip],
                        ones64[:],
                        recb[:, 0:ip],
                        start=True,
                        stop=True,
                    )
                    rb = work.tile([DH, ip], F32, tag="rb")
                    nc.vector.tensor_copy(rb[:, 0:ip], bps[:, 0:ip])
                    nc.vector.tensor_tensor(
                        OT_sb[hh * DH : (hh + 1) * DH, hp, 0:ip],
                        oacc[0:DH, 0:ip],
                        rb[:, 0:ip],
                        mybir.AluOpType.mult,
                    )

            b_all()
            for ep in range(EC // 2):
                a_chunk(ep)
                c_pair(2 * ep)
                c_pair(2 * ep + 1)

            wo_sb = wslot.tile([128, DC, E], BF16, tag="w")
            nc.gpsimd.dma_start(
                wo_sb[:], wout[:].rearrange("(ec p) d -> p ec d", p=128)
            )
            for ic in range(IC):
                has_valid = ic * 128 < ip
                if has_valid:
                    yps = ppool.tile([128, 2, 512], F32, tag="s2")
                    for db in range(2):
                        for ec in range(EC):
                            nc.tensor.matmul(
                                yps[:, db, :],
                                OT_sb[:, ec, ic * 128 : (ic + 1) * 128],
                                wo_sb[:, ec, db * 512 : (db + 1) * 512],
                                start=(ec == 0),
                                stop=(ec == EC - 1),
                            )
                bb_a = accp.tile([128, 512], F32, tag="oacc")
                bb_b = accp.tile([128, 512], F32, tag="oacc")
                for db, bbps in ((0, bb_a), (1, bb_b)):
                    nc.tensor.matmul(
                        bbps[:],
                        omqrow_sb[:, ic * 128 : (ic + 1) * 128],
                        ymeanb_sb[:, db * 512 : (db + 1) * 512],
                        start=True,
                        stop=False,
                    )
                    nc.tensor.matmul(
                        bbps[:],
                        qmrow_sb[:, ic * 128 : (ic + 1) * 128],
                        boutr_sb[:, db * 512 : (db + 1) * 512],
                        start=False,
                        stop=True,
                    )
                y1 = work.tile([128, 2, 512], F32, tag="y1")
                if has_valid:
                    nc.vector.tensor_scalar_mul(
                        y1[:], yps[:], qmp_sb[:, ic : ic + 1]
                    )
                    nc.vector.tensor_tensor(
                        y1[:, 0, :], bb_a[:], y1[:, 0, :], mybir.AluOpType.add
                    )
                    nc.vector.tensor_tensor(
                        y1[:, 1, :], bb_b[:], y1[:, 1, :], mybir.AluOpType.add
                    )
                else:
                    nc.vector.tensor_copy(y1[:, 0, :], bb_a[:])
                    nc.vector.tensor_copy(y1[:, 1, :], bb_b[:])
                nc.sync.dma_start(
                    y[ic * 128 : (ic + 1) * 128, :],
                    y1[:].rearrange("p b d -> p (b d)"),
                )

    nc.compile()
    return nc


_NC_CACHE = {}


def _get_nc(jp=J, ip=I):
    key = (jp, ip)
    if key not in _NC_CACHE:
        build = build_nc if jp <= JP_RESIDENT_MAX else build_nc_stream
        _NC_CACHE[key] = build(jp, ip)
    return _NC_CACHE[key]


def prep_inputs(txt, image, kv_mask, q_mask, Wq, Wkv, Wout, bout):
    f32 = np.float32
    Wq = np.asarray(Wq, dtype=f32)
    Wkv = np.asarray(Wkv, dtype=f32)
    Wout = np.asarray(Wout, dtype=f32)
    bout = np.asarray(bout, dtype=f32)
    wq_b = Wq.astype(BF)
    wkv_b = Wkv.astype(BF)
    wout_b = Wout.astype(BF)
    kvc = kv_mask.sum(axis=1).max()
    qc = q_mask.sum(axis=1).max()
    jp = max(512, int(-(-kvc // 128)) * 128)
    ip = max(256, int(-(-qc // 16)) * 16)
    jcp = jp // 128
    in_maps = []
    perms = []
    for b in range(B):
        kvm = kv_mask[b].astype(bool)
        qm = q_mask[b].astype(bool)
        nkv = int(kvm.sum())
        # compact image columns to valid kv positions, zero-pad to jp
        imTc = np.zeros((D, jp), dtype=BF)
        imTc[:, :nkv] = np.ascontiguousarray(image[b][kvm].T).astype(BF)
        kvmp = np.zeros(jp, dtype=f32)
        kvmp[:nkv] = 1.0
        # permute txt rows valid-first
        perm = np.argsort(~qm, kind="stable")
        perms.append(perm)
        qmperm = qm[perm].astype(f32)
        xmean = image[b].astype(f32).mean(axis=0)
        vmean = xmean @ Wkv[:, E:]
        ymb = vmean @ Wout + bout
        in_maps.append(
            {
                "txtT": np.ascontiguousarray(txt[b][perm].T).astype(BF),
                "imT": imTc,
                "wq": wq_b,
                "wkv": wkv_b,
                "wout": wout_b,
                "kvmp": np.ascontiguousarray(kvmp.reshape(jcp, 128).T),
                "qmp": np.ascontiguousarray(qmperm.reshape(IC, 128).T),
                "qmrow": qmperm[None, :].astype(BF),
                "omqrow": (1.0 - qmperm)[None, :].astype(BF),
                "ymeanb": ymb[None, :].astype(BF),
                "boutr": bout[None, :].astype(BF),
            }
        )
    return in_maps, perms, jp, ip


def run(inputs, trace=False):
    in_maps, perms, jp, ip = prep_inputs(**inputs)
    nc = _get_nc(jp, ip)
    res = run_bass_kernel_spmd(
        nc, in_maps, core_ids=list(range(B)), trace=trace,
        **({"trace_cores": [0]} if trace else {}),
    )
    out = np.empty((B, I, D), dtype=np.float32)
    for b in range(B):
        out[b][perms[b]] = res.results[b]["y"]
    return out, res


def kernel(**inputs):
    out, _ = run(inputs, trace=False)
    return out


# revision 16
# speedup vs baseline: 2.1057x; 1.9111x over previous
"""Cross-attention (txt queries -> image kv) Trainium2 Bass kernel.

Sharding: data-parallel over batch — B=8 batches, one NeuronCore each.
Host-side prep: image columns are COMPACTED to valid kv positions (padded to
jp, a multiple of 256) and txt rows PERMUTED valid-first (attention runs on
the first ip columns only; outputs un-permuted on host). Invalid-q rows are
reconstructed exactly via the ymeanb blend (uniform attention over all kv).
Per core (batch b):
  Q^T = Wq^T T^T / 8          [e, i]   bf16 SBUF-resident
  K^T = Wk^T X^T              [e, j]   bf16 SBUF-resident
  V'  = (X Wv) * kvm_j        [j, e]   bf16 SBUF-resident; per head tiles
                                       [j, h, 65] whose col 64 holds kvm_j
                                       (so PV accumulates both numerator and
                                       softmax denominator with the kv mask
                                       applied exactly)
  S^T_h = K_h Q_h^T           [j, i]   psum, two heads per [128,2,512] tile
  P^T = exp(S^T * kvm_j)      (ACT scale=kvm; masked/padded rows give exp(0)=1
                               but are zeroed by V' — no separate mask op)
  O^T_aug_h = [V'_h | kvm] ^T P^T -> [65, i] psum accum over jchunks
  O^T = O^T_aug[0:64] * recip(denom) broadcast via PE ones-outer-product
  Y = O^T.T Wout; blend: y = qm_i*Y + (1-qm_i)*ymeanb + qm_i*bout
    (q_mask=False rows = uniform attention over all kv -> host-computed
     ymeanb = (mean_j X) @ Wv @ Wout + bout)

Fast path (jp <= 2816): X^T kept fully SBUF-resident — read from HBM exactly
once (the old streaming path re-read it 5x).  K^T chunk computation (PE) is
software-pipelined against the previous chunk's attention (QK^T/exp/PV), so
the Activation engine's exp work hides behind PE matmuls, and within the
attention inner loop QK^T(jc+1) is issued ahead of PV(jc) so PE never waits
on the exp of the current chunk.

All PE matmuls in bf16 (inputs pre-cast to bf16 on host; intermediates cast
during psum->sbuf copies). Accumulation is fp32 in PSUM throughout.
"""

import ml_dtypes
import numpy as np

import concourse.bass as bass
from concourse import bacc
import concourse.mybir as mybir
import concourse.tile as tile
from concourse.bass_utils import run_bass_kernel_spmd

B, I, J, D, E = 8, 512, 4096, 1024, 1024
H, DH = 16, 64
JC = J // 128   # 32
IC = I // 128   # 4
DC = D // 128   # 8
EC = E // 128   # 8
F32 = mybir.dt.float32
BF16 = mybir.dt.bfloat16
BF = ml_dtypes.bfloat16

JP_RESIDENT_MAX = 2816  # X^T fully SBUF-resident up to this padded kv length


def _mix(*streams):
    """Proportionally interleave step generators.

    Each stream is (generator, approx_step_count).  Steps are drawn so every
    stream's completed fraction stays balanced — this sets the PE program
    order so K^T-projection matmuls fill the gaps the attention loop leaves
    while it waits on Activation-engine exps."""
    live = [[g, n, 0] for g, n in streams if n > 0]
    while live:
        g, n, done = min(live, key=lambda s: s[2] / s[1])
        try:
            next(g)
            for s in live:
                if s[0] is g:
                    s[2] += 1
        except StopIteration:
            live = [s for s in live if s[0] is not g]


def build_nc(jp=J, ip=I):
    jcp = jp // 128
    nc = bacc.Bacc()

    txtT = nc.dram_tensor("txtT", [D, I], BF16, kind="ExternalInput")
    imT = nc.dram_tensor("imT", [D, jp], BF16, kind="ExternalInput")
    wq = nc.dram_tensor("wq", [D, E], BF16, kind="ExternalInput")
    wkv = nc.dram_tensor("wkv", [D, 2 * E], BF16, kind="ExternalInput")
    wout = nc.dram_tensor("wout", [E, D], BF16, kind="ExternalInput")
    kvmp = nc.dram_tensor("kvmp", [128, jcp], F32, kind="ExternalInput")
    qmp = nc.dram_tensor("qmp", [128, IC], F32, kind="ExternalInput")
    qmrow = nc.dram_tensor("qmrow", [1, I], BF16, kind="ExternalInput")
    omqrow = nc.dram_tensor("omqrow", [1, I], BF16, kind="ExternalInput")
    ymeanb = nc.dram_tensor("ymeanb", [1, D], BF16, kind="ExternalInput")
    boutr = nc.dram_tensor("boutr", [1, D], BF16, kind="ExternalInput")
    y = nc.dram_tensor("y", [I, D], BF16, kind="ExternalOutput")

    jblocks = []
    off = 0
    while off < jp:
        w = 512 if jp - off >= 512 else jp - off
        jblocks.append((off, w))
        off += w

    with tile.TileContext(nc) as tc:
        with (
            tc.tile_pool(name="wpool", bufs=1) as wpool,      # small resident
            tc.tile_pool(name="wslot", bufs=3) as wslot,      # wq/wv/wk live together
            tc.tile_pool(name="big", bufs=1) as big,          # imr, K^T, V, Q^T, O^T
            tc.tile_pool(name="work", bufs=3) as work,        # small working tiles
            tc.tile_pool(name="etp", bufs=4) as etp,
            tc.tile_pool(name="ppool", bufs=3, space="PSUM") as ppool,   # 6 banks
            tc.tile_pool(name="accp", bufs=2, space="PSUM") as accp,     # 2 banks
        ):
            # phase Q's operands go first — every DMA transfer serializes
            # through the DMA engines, so these two gate PE start
            ta = wpool.tile([128, DC, I], BF16)
            tar = txtT[:].rearrange("(dc p) i -> p dc i", p=128)
            wqr = wq[:].rearrange("(dc p) e -> p dc e", p=128)
            wq_sb = wslot.tile([128, DC, E], BF16, tag="w")
            hdc = DC // 2
            nc.sync.dma_start(ta[:, 0:hdc, :], tar[:, 0:hdc, :])
            nc.scalar.dma_start(wq_sb[:, 0:hdc, :], wqr[:, 0:hdc, :])
            nc.sync.dma_start(ta[:, hdc:DC, :], tar[:, hdc:DC, :])
            nc.scalar.dma_start(wq_sb[:, hdc:DC, :], wqr[:, hdc:DC, :])

            # ---- resident small inputs ----
            kvm_sb = wpool.tile([128, jcp], F32)
            nc.sync.dma_start(kvm_sb[:], kvmp[:])
            qmp_sb = wpool.tile([128, IC], F32)
            nc.sync.dma_start(qmp_sb[:], qmp[:])
            qmrow_sb = wpool.tile([1, I], BF16)
            nc.sync.dma_start(qmrow_sb[:], qmrow[:])
            omqrow_sb = wpool.tile([1, I], BF16)
            nc.sync.dma_start(omqrow_sb[:], omqrow[:])
            ymeanb_sb = wpool.tile([1, D], BF16)
            nc.sync.dma_start(ymeanb_sb[:], ymeanb[:])
            boutr_sb = wpool.tile([1, D], BF16)
            nc.sync.dma_start(boutr_sb[:], boutr[:])
            ones64 = wpool.tile([1, DH], BF16)
            nc.vector.memset(ones64[:], 1.0)

            # ---- big residents: X^T once from HBM, K^T/V/Q^T/O^T on-chip ----
            # DMA order is the startup critical path: phase Q needs wq+ta
            # first, then V needs wv+imr, then a_chunk(0) needs wk.  imr is
            # loaded block-by-block so phase V's first chunks don't wait for
            # the full 4.5MB transfer (subtile deps track the slices).
            imr = big.tile([128, DC, jp], BF16)
            KT_sb = big.tile([128, EC, jp], BF16)
            QT_sb = big.tile([128, EC, I], BF16)
            OT_sb = big.tile([128, EC, I], BF16)
            nc.vector.memset(OT_sb[:], 0.0)
            V_sb = big.tile([128, jcp, H, DH + 1], BF16)
            nc.vector.tensor_copy(
                V_sb[:, :, :, DH : DH + 1],
                kvm_sb[:, :, None, None].to_broadcast([128, jcp, H, 1]),
            )

            wv_sb = wslot.tile([128, DC, E], BF16, tag="w")
            nc.scalar.dma_start(
                wv_sb[:], wkv[:, E : 2 * E].rearrange("(dc p) e -> p dc e", p=128)
            )
            imTr = imT[:].rearrange("(dc p) j -> p dc j", p=128)
            for off, w in [(o, min(512, jp - o)) for o in range(0, jp, 512)]:
                nc.scalar.dma_start(
                    imr[:, :, off : off + w], imTr[:, :, off : off + w]
                )
            wk_sb = wslot.tile([128, DC, E], BF16, tag="w")
            nc.scalar.dma_start(
                wk_sb[:], wkv[:, 0:E].rearrange("(dc p) e -> p dc e", p=128)
            )

            # ============ Phase Q: Q^T = Wq^T @ T^T, scaled 1/8 ==========
            for ep in range(EC // 2):
                ps = ppool.tile([128, 2, 512], F32, tag="sps", bufs=2)
                for eh in range(2):
                    ec = 2 * ep + eh
                    for dc in range(DC):
                        nc.tensor.matmul(
                            ps[:, eh, 0:ip],
                            wq_sb[:, dc, ec * 128 : (ec + 1) * 128],
                            ta[:, dc, 0:ip],
                            start=(dc == 0),
                            stop=(dc == DC - 1),
                        )
                nc.vector.tensor_scalar_mul(
                    QT_sb[:, 2 * ep : 2 * ep + 2, 0:ip], ps[:, :, 0:ip], 0.125
                )

            # ============ Phase V: V' = (X Wv) * kvm, from resident X^T ====
            def v_phase():
                for jc in range(jcp):
                    ps = ppool.tile([128, 2, 512], F32, tag="sps", bufs=2)
                    for eb in range(2):
                        for dc in range(DC):
                            nc.tensor.matmul(
                                ps[:, eb, :],
                                imr[:, dc, jc * 128 : (jc + 1) * 128],
                                wv_sb[:, dc, eb * 512 : (eb + 1) * 512],
                                start=(dc == 0),
                                stop=(dc == DC - 1),
                            )
                    nc.vector.tensor_scalar_mul(
                        V_sb[:, jc, :, 0:DH],
                        ps[:].rearrange("p b (h dh) -> p (b h) dh", dh=DH),
                        kvm_sb[:, jc : jc + 1],
                    )
                    yield

            # ====== Main loop: K^T chunks pipelined against attention ======
            def a_chunk(ep):
                # K^T rows for ec chunks 2ep, 2ep+1, from resident X^T
                for off, w in jblocks:
                    ps = ppool.tile([128, 2, 512], F32, tag="aps", bufs=1)
                    for eh in range(2):
                        ec = 2 * ep + eh
                        for dc in range(DC):
                            nc.tensor.matmul(
                                ps[:, eh, 0:w],
                                wk_sb[:, dc, ec * 128 : (ec + 1) * 128],
                                imr[:, dc, off : off + w],
                                start=(dc == 0),
                                stop=(dc == DC - 1),
                            )
                        yield
                    nc.vector.tensor_copy(
                        KT_sb[:, 2 * ep : 2 * ep + 2, off : off + w],
                        ps[:, :, 0:w],
                    )

            def c_pair(hp, accs=None, lookahead=2):
                vt = V_sb[:, :, 2 * hp : 2 * hp + 2, :]
                if accs is None:
                    oacc_a = accp.tile([DH + 1, 512], F32, tag="oacc")
                    oacc_b = accp.tile([DH + 1, 512], F32, tag="oacc")
                else:
                    oacc_a, oacc_b = accs

                def qk(jc):
                    sps = ppool.tile([128, 2, 512], F32, tag="sps", bufs=2)
                    nc.tensor.matmul(
                        sps[:, 0, 0:ip],
                        KT_sb[0:DH, hp, jc * 128 : (jc + 1) * 128],
                        QT_sb[0:DH, hp, 0:ip],
                        start=True,
                        stop=True,
                    )
                    nc.tensor.matmul(
                        sps[:, 1, 0:ip],
                        KT_sb[DH:128, hp, jc * 128 : (jc + 1) * 128],
                        QT_sb[DH:128, hp, 0:ip],
                        start=True,
                        stop=True,
                    )
                    et = etp.tile([128, 2, ip], BF16, tag="et")
                    nc.scalar.activation(
                        et[:, :, 0:ip],
                        sps[:, :, 0:ip],
                        mybir.ActivationFunctionType.Exp,
                        scale=kvm_sb[:, jc : jc + 1],
                    )
                    return et

                def pv(jc, et):
                    nc.tensor.matmul(
                        oacc_a[:, 0:ip],
                        vt[:, jc, 0, :],
                        et[:, 0, 0:ip],
                        start=(jc == 0),
                        stop=(jc == jcp - 1),
                    )
                    nc.tensor.matmul(
                        oacc_b[:, 0:ip],
                        vt[:, jc, 1, :],
                        et[:, 1, 0:ip],
                        start=(jc == 0),
                        stop=(jc == jcp - 1),
                    )

                # software pipeline: QK^T issued `lookahead` chunks ahead of
                # PV so PE never waits on the Activation engine's exp
                ets = []
                for jc in range(lookahead):
                    ets.append(qk(jc))
                    yield
                for jc in range(lookahead, jcp):
                    ets.append(qk(jc))
                    pv(jc - lookahead, ets.pop(0))
                    yield
                for k in range(lookahead):
                    pv(jcp - lookahead + k, ets.pop(0))
                    if k + 1 < lookahead:
                        yield

                # denominator reciprocals first (DVE), then the PE broadcasts —
                # with yields between so interleaved a_chunk matmuls cover the
                # DVE latency
                recbs = []
                for hh, oacc in ((0, oacc_a), (1, oacc_b)):
                    rec = work.tile([1, ip], F32, tag="rec")
                    nc.vector.reciprocal(rec[:, 0:ip], oacc[DH : DH + 1, 0:ip])
                    recb = work.tile([1, ip], BF16, tag="recb")
                    nc.vector.tensor_copy(recb[:, 0:ip], rec[:, 0:ip])
                    recbs.append(recb)
                yield
                for hh, oacc in ((0, oacc_a), (1, oacc_b)):
                    bps = ppool.tile([128, 2, 512], F32, tag="sps", bufs=2)
                    nc.tensor.matmul(
                        bps[0:DH, 0, 0:ip],
                        ones64[:],
                        recbs[hh][:, 0:ip],
                        start=True,
                        stop=True,
                    )
                    rb = work.tile([DH, ip], F32, tag="rb")
                    nc.vector.tensor_copy(rb[:, 0:ip], bps[0:DH, 0, 0:ip])
                    nc.vector.tensor_tensor(
                        OT_sb[hh * DH : (hh + 1) * DH, hp, 0:ip],
                        oacc[0:DH, 0:ip],
                        rb[:, 0:ip],
                        mybir.AluOpType.mult,
                    )
                    yield

            # prefetch the output-projection weights into wq's slot (its
            # readers are done after phase Q, so this overlaps the main loop)
            wo_sb = wslot.tile([128, DC, E], BF16, tag="w")
            nc.gpsimd.dma_start(
                wo_sb[:], wout[:].rearrange("(ec p) d -> p ec d", p=128)
            )

            def c_chain(hp0):
                # the two attention pairs for one K^T chunk, back-to-back
                # (sequential so only one pair's PSUM accumulators are live)
                yield from c_pair(hp0)
                yield from c_pair(hp0 + 1)

            n_a = 2 * len(jblocks)        # yields per a_chunk
            n_c = 2 * (jcp + 4)           # yields per c_chain
            _mix((v_phase(), jcp), (a_chunk(0), n_a))
            for ep in range(1, EC // 2):
                _mix((a_chunk(ep), n_a), (c_chain(2 * ep - 2), n_c))
            # drain the last two attention pairs, interleaved: the second
            # pair's accumulators borrow the a_chunk PSUM slot (idle now), and
            # lookahead drops to 1 so two pairs fit the sps rotation
            tacc = ppool.tile([128, 2, 512], F32, tag="aps", bufs=1)
            _mix(
        (c_pair(EC - 2, lookahead=1), jcp + 3),
        (c_pair(EC - 1, accs=(tacc[0 : DH + 1, 0, :], tacc[0 : DH + 1, 1, :]),
                lookahead=1), jcp + 3),
            )

            # ============ Phase D: Y = O^T.T @ Wout + blend ==============
            ic_order = sorted(range(IC), key=lambda ic: ic * 128 < ip)
            for ic in ic_order:
                has_valid = ic * 128 < ip
                if has_valid:
                    yps = ppool.tile([128, 2, 512], F32, tag="sps", bufs=2)
                    for db in range(2):
                        for ec in range(EC):
                            nc.tensor.matmul(
                                yps[:, db, :],
                                OT_sb[:, ec, ic * 128 : (ic + 1) * 128],
                                wo_sb[:, ec, db * 512 : (db + 1) * 512],
                                start=(ec == 0),
                                stop=(ec == EC - 1),
                            )
                bb_a = accp.tile([128, 512], F32, tag="oacc")
                bb_b = accp.tile([128, 512], F32, tag="oacc")
                for db, bbps in ((0, bb_a), (1, bb_b)):
                    nc.tensor.matmul(
                        bbps[:],
                        omqrow_sb[:, ic * 128 : (ic + 1) * 128],
                        ymeanb_sb[:, db * 512 : (db + 1) * 512],
                        start=True,
                        stop=False,
                    )
                    nc.tensor.matmul(
                        bbps[:],
                        qmrow_sb[:, ic * 128 : (ic + 1) * 128],
                        boutr_sb[:, db * 512 : (db + 1) * 512],
                        start=False,
                        stop=True,
                    )
                # per-half blend + DMA so the first half's store overlaps
                # the second half's vector work
                for db, bb in ((0, bb_a), (1, bb_b)):
                    y1 = work.tile([128, 512], BF16, tag="y1", bufs=3)
                    if has_valid:
                        nc.vector.tensor_scalar_mul(
                            y1[:], yps[:, db, :], qmp_sb[:, ic : ic + 1]
                        )
                        nc.vector.tensor_tensor(
                            y1[:], bb[:], y1[:], mybir.AluOpType.add
                        )
                    else:
                        nc.vector.tensor_copy(y1[:], bb[:])
                    nc.sync.dma_start(
                        y[ic * 128 : (ic + 1) * 128, db * 512 : (db + 1) * 512],
                        y1[:],
                    )

    nc.compile()
    return nc


def build_nc_stream(jp=J, ip=I):
    """Streaming fallback for jp > JP_RESIDENT_MAX (X^T doesn't fit SBUF):
    the original 5-pass streaming kernel, V through a DRAM round-trip."""
    jcp = jp // 128
    nc = bacc.Bacc()

    txtT = nc.dram_tensor("txtT", [D, I], BF16, kind="ExternalInput")
    imT = nc.dram_tensor("imT", [D, jp], BF16, kind="ExternalInput")
    wq = nc.dram_tensor("wq", [D, E], BF16, kind="ExternalInput")
    wkv = nc.dram_tensor("wkv", [D, 2 * E], BF16, kind="ExternalInput")
    wout = nc.dram_tensor("wout", [E, D], BF16, kind="ExternalInput")
    kvmp = nc.dram_tensor("kvmp", [128, jcp], F32, kind="ExternalInput")
    qmp = nc.dram_tensor("qmp", [128, IC], F32, kind="ExternalInput")
    qmrow = nc.dram_tensor("qmrow", [1, I], BF16, kind="ExternalInput")
    omqrow = nc.dram_tensor("omqrow", [1, I], BF16, kind="ExternalInput")
    ymeanb = nc.dram_tensor("ymeanb", [1, D], BF16, kind="ExternalInput")
    boutr = nc.dram_tensor("boutr", [1, D], BF16, kind="ExternalInput")
    y = nc.dram_tensor("y", [I, D], F32, kind="ExternalOutput")
    vdr = nc.dram_tensor("vdr", [jcp, 128, H, DH], BF16, kind="Internal")

    imTr = imT[:].rearrange("(dc p) j -> p dc j", p=128)

    with tile.TileContext(nc) as tc:
        with (
            tc.tile_pool(name="wpool", bufs=1) as wpool,
            tc.tile_pool(name="wslot", bufs=1) as wslot,
            tc.tile_pool(name="big", bufs=1) as big,
            tc.tile_pool(name="stream", bufs=2) as stream,
            tc.tile_pool(name="work", bufs=3) as work,
            tc.tile_pool(name="etp", bufs=7) as etp,
            tc.tile_pool(name="ppool", bufs=2, space="PSUM") as ppool,
            tc.tile_pool(name="accp", bufs=3, space="PSUM") as accp,
            tc.tile_pool(name="bcp", bufs=1, space="PSUM") as bcp,
        ):
            kvm_sb = wpool.tile([128, jcp], F32)
            nc.sync.dma_start(kvm_sb[:], kvmp[:])
            qmp_sb = wpool.tile([128, IC], F32)
            nc.sync.dma_start(qmp_sb[:], qmp[:])
            qmrow_sb = wpool.tile([1, I], BF16)
            nc.sync.dma_start(qmrow_sb[:], qmrow[:])
            omqrow_sb = wpool.tile([1, I], BF16)
            nc.sync.dma_start(omqrow_sb[:], omqrow[:])
            ymeanb_sb = wpool.tile([1, D], BF16)
            nc.sync.dma_start(ymeanb_sb[:], ymeanb[:])
            boutr_sb = wpool.tile([1, D], BF16)
            nc.sync.dma_start(boutr_sb[:], boutr[:])
            ones64 = wpool.tile([1, DH], BF16)
            nc.vector.memset(ones64[:], 1.0)

            KT_sb = big.tile([128, EC, jp], BF16)
            QT_sb = big.tile([128, EC, I], BF16)
            OT_sb = big.tile([128, EC, I], BF16)
            nc.vector.memset(OT_sb[:], 0.0)

            wq_sb = wslot.tile([128, DC, E], BF16, tag="w")
            nc.gpsimd.dma_start(wq_sb[:], wq[:].rearrange("(dc p) e -> p dc e", p=128))
            ta = stream.tile([128, DC, I], BF16, tag="im")
            nc.gpsimd.dma_start(ta[:], txtT[:].rearrange("(dc p) i -> p dc i", p=128))
            for ep in range(EC // 2):
                ps = ppool.tile([128, 2, 512], F32, tag="s2")
                for eh in range(2):
                    ec = 2 * ep + eh
                    for dc in range(DC):
                        nc.tensor.matmul(
                            ps[:, eh, 0:ip],
                            wq_sb[:, dc, ec * 128 : (ec + 1) * 128],
                            ta[:, dc, 0:ip],
                            start=(dc == 0),
                            stop=(dc == DC - 1),
                        )
                nc.vector.tensor_scalar_mul(
                    QT_sb[:, 2 * ep : 2 * ep + 2, 0:ip], ps[:, :, 0:ip], 0.125
                )

            wv_sb = wslot.tile([128, DC, E], BF16, tag="w")
            nc.gpsimd.dma_start(
                wv_sb[:], wkv[:, E : 2 * E].rearrange("(dc p) e -> p dc e", p=128)
            )
            wk_sb = wslot.tile([128, DC, E], BF16, tag="w")
            nc.gpsimd.dma_start(
                wk_sb[:], wkv[:, 0:E].rearrange("(dc p) e -> p dc e", p=128)
            )

            jblocks = []
            off = 0
            while off < jp:
                w = 512 if jp - off >= 512 else jp - off
                jblocks.append((off, w))
                off += w

            def b_all():
                for off, w in jblocks:
                    imb = stream.tile([128, DC, 512], BF16, tag="ima")
                    nc.gpsimd.dma_start(
                        imb[:, :, 0:w], imTr[:, :, off : off + w]
                    )
                    for jh in range(w // 128):
                        jc = off // 128 + jh
                        ps = ppool.tile([128, 2, 512], F32, tag="s2")
                        for eb in range(2):
                            for dc in range(DC):
                                nc.tensor.matmul(
                                    ps[:, eb, :],
                                    imb[:, dc, jh * 128 : (jh + 1) * 128],
                                    wv_sb[:, dc, eb * 512 : (eb + 1) * 512],
                                    start=(dc == 0),
                                    stop=(dc == DC - 1),
                                )
                        vtmp = work.tile([128, H, DH], BF16, tag="vtmp")
                        nc.vector.tensor_scalar_mul(
                            vtmp[:],
                            ps[:].rearrange("p b (h dh) -> p (b h) dh", dh=DH),
                            kvm_sb[:, jc : jc + 1],
                        )
                        nc.sync.dma_start(vdr[jc, :, :, :], vtmp[:])

            def a_chunk(ep):
                for off, w in jblocks:
                    ima = stream.tile([128, DC, 512], BF16, tag="ima")
                    nc.gpsimd.dma_start(ima[:, :, 0:w], imTr[:, :, off : off + w])
                    ps = ppool.tile([128, 2, 512], F32, tag="s2")
                    for eh in range(2):
                        ec = 2 * ep + eh
                        for dc in range(DC):
                            nc.tensor.matmul(
                                ps[:, eh, 0:w],
                                wk_sb[:, dc, ec * 128 : (ec + 1) * 128],
                                ima[:, dc, 0:w],
                                start=(dc == 0),
                                stop=(dc == DC - 1),
                            )
                    nc.vector.tensor_copy(
                        KT_sb[:, 2 * ep : 2 * ep + 2, off : off + w],
                        ps[:, :, 0:w],
                    )

            def c_pair(hp):
                vtt = stream.tile([128, jcp, 2, DH + 1], BF16, tag="vt")
                nc.vector.tensor_copy(
                    vtt[:, :, 0, DH : DH + 1], kvm_sb[:, :, None]
                )
                nc.vector.tensor_copy(
                    vtt[:, :, 1, DH : DH + 1], kvm_sb[:, :, None]
                )
                for hh in range(2):
                    nc.sync.dma_start(
                        vtt[:, :, hh, 0:DH],
                        vdr[:, :, 2 * hp + hh, :].rearrange("jc p dh -> p jc dh"),
                    )
                vt = vtt
                oacc_a = accp.tile([DH + 1, 512], F32, tag="oacc")
                oacc_b = accp.tile([DH + 1, 512], F32, tag="oacc")
                for jc in range(jcp):
                    sps = ppool.tile([128, 2, 512], F32, tag="s2")
                    nc.tensor.matmul(
                        sps[:, 0, 0:ip],
                        KT_sb[0:DH, hp, jc * 128 : (jc + 1) * 128],
                        QT_sb[0:DH, hp, 0:ip],
                        start=True,
                        stop=True,
                    )
                    nc.tensor.matmul(
                        sps[:, 1, 0:ip],
                        KT_sb[DH:128, hp, jc * 128 : (jc + 1) * 128],
                        QT_sb[DH:128, hp, 0:ip],
                        start=True,
                        stop=True,
                    )
                    et = etp.tile([128, 2, ip], BF16, tag="et")
                    nc.scalar.activation(
                        et[:, :, 0:ip],
                        sps[:, :, 0:ip],
                        mybir.ActivationFunctionType.Exp,
                        scale=kvm_sb[:, jc : jc + 1],
                    )
                    nc.tensor.matmul(
                        oacc_a[:, 0:ip],
                        vt[:, jc, 0, :],
                        et[:, 0, 0:ip],
                        start=(jc == 0),
                        stop=(jc == jcp - 1),
                    )
                    nc.tensor.matmul(
                        oacc_b[:, 0:ip],
                        vt[:, jc, 1, :],
                        et[:, 1, 0:ip],
                        start=(jc == 0),
                        stop=(jc == jcp - 1),
                    )
                for hh, oacc in ((0, oacc_a), (1, oacc_b)):
                    rec = work.tile([1, ip], F32, tag="rec")
                    nc.vector.reciprocal(rec[:, 0:ip], oacc[DH : DH + 1, 0:ip])
                    recb = work.tile([1, ip], BF16, tag="recb")
                    nc.vector.tensor_copy(recb[:, 0:ip], rec[:, 0:ip])
                    bps = bcp.tile([DH, 512], F32, tag="bc")
                    nc.tensor.matmul(
                        bps[:, 0:ip],
                        ones64[:],
                        recb[:, 0:ip],
                        start=True,
                        stop=True,
                    )
                    rb = work.tile([DH, ip], F32, tag="rb")
                    nc.vector.tensor_copy(rb[:, 0:ip], bps[:, 0:ip])
                    nc.vector.tensor_tensor(
                        OT_sb[hh * DH : (hh + 1) * DH, hp, 0:ip],
                        oacc[0:DH, 0:ip],
                        rb[:, 0:ip],
                        mybir.AluOpType.mult,
                    )

            b_all()
            for ep in range(EC // 2):
                a_chunk(ep)
                c_pair(2 * ep)
                c_pair(2 * ep + 1)

            wo_sb = wslot.tile([128, DC, E], BF16, tag="w")
            nc.gpsimd.dma_start(
                wo_sb[:], wout[:].rearrange("(ec p) d -> p ec d", p=128)
            )
            for ic in range(IC):
                has_valid = ic * 128 < ip
                if has_valid:
                    yps = ppool.tile([128, 2, 512], F32, tag="s2")
                    for db in range(2):
                        for ec in range(EC):
                            nc.tensor.matmul(
                                yps[:, db, :],
                                OT_sb[:, ec, ic * 128 : (ic + 1) * 128],
                                wo_sb[:, ec, db * 512 : (db + 1) * 512],
                                start=(ec == 0),
                                stop=(ec == EC - 1),
                            )
                bb_a = accp.tile([128, 512], F32, tag="oacc")
                bb_b = accp.tile([128, 512], F32, tag="oacc")
                for db, bbps in ((0, bb_a), (1, bb_b)):
                    nc.tensor.matmul(
                        bbps[:],
                        omqrow_sb[:, ic * 128 : (ic + 1) * 128],
                        ymeanb_sb[:, db * 512 : (db + 1) * 512],
                        start=True,
                        stop=False,
                    )
                    nc.tensor.matmul(
                        bbps[:],
                        qmrow_sb[:, ic * 128 : (ic + 1) * 128],
                        boutr_sb[:, db * 512 : (db + 1) * 512],
                        start=False,
                        stop=True,
                    )
                y1 = work.tile([128, 2, 512], F32, tag="y1")
                if has_valid:
                    nc.vector.tensor_scalar_mul(
                        y1[:], yps[:], qmp_sb[:, ic : ic + 1]
                    )
                    nc.vector.tensor_tensor(
                        y1[:, 0, :], bb_a[:], y1[:, 0, :], mybir.AluOpType.add
                    )
                    nc.vector.tensor_tensor(
                        y1[:, 1, :], bb_b[:], y1[:, 1, :], mybir.AluOpType.add
                    )
                else:
                    nc.vector.tensor_copy(y1[:, 0, :], bb_a[:])
                    nc.vector.tensor_copy(y1[:, 1, :], bb_b[:])
                nc.sync.dma_start(
                    y[ic * 128 : (ic + 1) * 128, :],
                    y1[:].rearrange("p b d -> p (b d)"),
                )

    nc.compile()
    return nc


_NC_CACHE = {}


def _get_nc(jp=J, ip=I):
    key = (jp, ip)
    if key not in _NC_CACHE:
        build = build_nc if jp <= JP_RESIDENT_MAX else build_nc_stream
        _NC_CACHE[key] = build(jp, ip)
    return _NC_CACHE[key]


def prep_inputs(txt, image, kv_mask, q_mask, Wq, Wkv, Wout, bout):
    f32 = np.float32
    Wq = np.asarray(Wq, dtype=f32)
    Wkv = np.asarray(Wkv, dtype=f32)
    Wout = np.asarray(Wout, dtype=f32)
    bout = np.asarray(bout, dtype=f32)
    wq_b = Wq.astype(BF)
    wkv_b = Wkv.astype(BF)
    wout_b = Wout.astype(BF)
    kvc = kv_mask.sum(axis=1).max()
    qc = q_mask.sum(axis=1).max()
    jp = max(512, int(-(-kvc // 128)) * 128)
    ip = max(256, int(-(-qc // 16)) * 16)
    jcp = jp // 128
    in_maps = []
    perms = []
    for b in range(B):
        kvm = kv_mask[b].astype(bool)
        qm = q_mask[b].astype(bool)
        nkv = int(kvm.sum())
        # compact image columns to valid kv positions, zero-pad to jp
        imTc = np.zeros((D, jp), dtype=BF)
        imTc[:, :nkv] = np.ascontiguousarray(image[b][kvm].T).astype(BF)
        kvmp = np.zeros(jp, dtype=f32)
        kvmp[:nkv] = 1.0
        # permute txt rows valid-first
        perm = np.argsort(~qm, kind="stable")
        perms.append(perm)
        qmperm = qm[perm].astype(f32)
        xmean = image[b].astype(f32).mean(axis=0)
        vmean = xmean @ Wkv[:, E:]
        ymb = vmean @ Wout + bout
        in_maps.append(
            {
                "txtT": np.ascontiguousarray(txt[b][perm].T).astype(BF),
                "imT": imTc,
                "wq": wq_b,
                "wkv": wkv_b,
                "wout": wout_b,
                "kvmp": np.ascontiguousarray(kvmp.reshape(jcp, 128).T),
                "qmp": np.ascontiguousarray(qmperm.reshape(IC, 128).T),
                "qmrow": qmperm[None, :].astype(BF),
                "omqrow": (1.0 - qmperm)[None, :].astype(BF),
                "ymeanb": ymb[None, :].astype(BF),
                "boutr": bout[None, :].astype(BF),
            }
        )
    return in_maps, perms, jp, ip


def run(inputs, trace=False):
    in_maps, perms, jp, ip = prep_inputs(**inputs)
    nc = _get_nc(jp, ip)
    res = run_bass_kernel_spmd(
        nc, in_maps, core_ids=list(range(B)), trace=trace,
        **({"trace_cores": [0]} if trace else {}),
    )
    out = np.empty((B, I, D), dtype=np.float32)
    for b in range(B):
        out[b][perms[b]] = res.results[b]["y"]
    return out, res


def kernel(**inputs):
    out, _ = run(inputs, trace=False)
    return out
